# revision 19
# baseline (speedup 1.0000x reference)
"""Trainium2 Bass kernel for single-step (decode) multi-head attention.

Module: y = o_proj(SDPA(q, K_cache<-k, V_cache<-v)) for B=16, S=1, D=2048,
H=16 heads, head_dim=128, KV cache length 4096, with the new k/v written at
`position` before attention.

Sharding: tensor-parallel over heads. 8 cores x 2 heads each. Each core gets
its slice of Wq/Wk/Wv rows (256 of 2048), Wo columns, and the K/V cache for
its 2 heads; it computes q/k/v projections, attention over the cache (with
the new k/v substituted at `position` on-device), and a partial o_proj.
The host sums the 8 cores' partial outputs.

Per-core DRAM layouts (pair p = local_head*16 + batch, 32 pairs/core), all
pre-packed on the host so every DMA lands contiguously per SBUF partition:
  kT: (32, 128, 4096)      K cache transposed -> (head_dim, kv) per pair
  v:  (32, 128, 32, 128)   V cache swizzled -> [pair, kv%128, kv//128, hd]
  xT/wqT/wkT/wvT/woT/yT:   (128, chunks, free) SBUF-image layouts

Scores per pair are 32 column matmuls (lhsT = kT 128x128 chunk, rhs = q
column) into a (128 kv, 32 chunk) PSUM tile; softmax is partition-parallel:
exp on ScalarE with fp32 accum_out row sums, per-pair totals via a
ones-vector matmul, normalization folded into the output scaling. The cache
update runs on-device: the new k column overwrites the stale kT column in
SBUF; on the V side the stale row's softmax weight is zeroed (one-hot
extract + mask) and the attn[position] * v_new term is added in fp32 in the
epilogue. The epilogue runs per head (head 0 at pair 15) and o_proj emits
the output transposed so all of yT fits one PSUM bank.

Precision: the cache-side matmuls (scores, attn @ V) and projections run in
bf16 (PE native single-pass dtype; fp32 matmuls cost ~3x via two half-rate
passes and doubled weight loads) with fp32 PSUM accumulation; softmax sums,
normalization, the new-token V term, and all reductions stay fp32.
Measured vs the fp32 reference: max-abs relative error ~4.6e-3, residual
variance ~2e-5. Set PRECISION = "fp32" for an exact (~3e-6) but ~4.3x
slower variant (fp32 everywhere).

Measured on 8 axon-tunneled trn2 NeuronCores: ~216-245 us HW exec
(DMA-bound: ~68 MB/core HBM traffic at ~360 GB/s/core + fixed barriers).
"""

import sys

for _p in ("/opt/trn_rl_repo", "/root/.axon_site/_ro/trn_rl_repo"):
    if _p not in sys.path:
        sys.path.append(_p)

import ml_dtypes
import numpy as np

import concourse.bacc as bacc
import concourse.mybir as mybir
import concourse.tile as tile
from concourse.bass_utils import run_bass_kernel_spmd

F32 = mybir.dt.float32
BF16 = mybir.dt.bfloat16

B = 16          # batch
D = 2048        # model dim
H_TOT = 16      # total heads
HD = 128        # head dim
KV = 4096       # cache length
N_CORES = 8
H_LOC = H_TOT // N_CORES       # 2 heads per core
PAIRS = H_LOC * B              # 32 (b,h) pairs per core
HS = H_LOC * HD                # 256-channel slice per core
DC = D // 128                  # 16 contraction chunks for projections

# Matches reference: scale = 1.0 / np.sqrt(head_dim).astype(np.float32)
SCALE = float(1.0 / np.sqrt(float(HD)).astype(np.float32))

PRECISION = "bf16"   # "bf16" (cache matmuls in bf16) or "fp32" (exact)

LAST_RESULT = None  # BassKernelResults of the most recent run (for profiling)


def build_kernel(position, kv=KV, prec=PRECISION):
    """Trace the per-core Bass kernel. `position` is baked in as a constant."""
    kvc = kv // 128              # number of 128-wide kv chunks
    pc, pi = position // 128, position % 128
    assert 0 <= position < kv
    CDT = BF16 if prec == "bf16" else F32

    nc = bacc.Bacc("TRN2", target_bir_lowering=False, debug=False)

    xT = nc.dram_tensor("xT", [128, DC, B], CDT, kind="ExternalInput").ap()
    wqT = nc.dram_tensor("wqT", [128, DC, HS], CDT, kind="ExternalInput").ap()
    wkT = nc.dram_tensor("wkT", [128, DC, HS], CDT, kind="ExternalInput").ap()
    wvT = nc.dram_tensor("wvT", [128, DC, HS], CDT, kind="ExternalInput").ap()
    woT = nc.dram_tensor("woT", [128, H_LOC, D], CDT, kind="ExternalInput").ap()
    kT = nc.dram_tensor("kT", [PAIRS, HD, kv], CDT, kind="ExternalInput").ap()
    v = nc.dram_tensor("v", [PAIRS, 128, kvc, HD], CDT, kind="ExternalInput").ap()
    yT = nc.dram_tensor("yT", [128, DC, B], F32, kind="ExternalOutput").ap()

    with tile.TileContext(nc) as tc:
        nbufs = 9 if prec == "bf16" else 3
        with (
            tc.tile_pool(name="wpool", bufs=1) as wpool,
            tc.tile_pool(name="spool", bufs=1) as spool,
            tc.tile_pool(name="kpool", bufs=nbufs) as kpool,
            tc.tile_pool(name="vpool", bufs=nbufs) as vpool,
            tc.tile_pool(name="ps_sc", bufs=3, space="PSUM") as ps_sc,
            tc.tile_pool(name="ps_one", bufs=1, space="PSUM") as ps_one,
        ):
            # ---- weights & x first on the fast sync ring (1.6 MB): they
            # gate the projections -> pair 0; on the SWDGE queue they get
            # starved by the cache-stream packets and land ~20 us late ----
            xT_sb = wpool.tile([128, DC, B], CDT)
            nc.sync.dma_start(xT_sb[:], xT)
            wq_sb = wpool.tile([128, DC, HS], CDT)
            nc.sync.dma_start(wq_sb[:], wqT)
            wk_sb = wpool.tile([128, DC, HS], CDT)
            nc.sync.dma_start(wk_sb[:], wkT)
            wv_sb = wpool.tile([128, DC, HS], CDT)
            nc.sync.dma_start(wv_sb[:], wvT)
            wo_sb = wpool.tile([128, H_LOC, D], CDT)

            # ---- then start the cache prefetch ----
            kts, vts = {}, {}

            def issue_pair_dma(p):
                kt = kpool.tile([128, kv], CDT, tag="kt")
                nc.sync.dma_start(kt[:], kT[p])
                kts[p] = kt
                vt = vpool.tile([128, kvc, HD], CDT, tag="vt")
                nc.sync.dma_start(vt[:], v[p])
                vts[p] = vt

            for _p0 in range(2):
                issue_pair_dma(_p0)

            # ---- constants ----
            ones_col = spool.tile([128, 1], F32)
            nc.vector.memset(ones_col[:], 1.0)
            ones_row = spool.tile([1, 128], F32)
            nc.vector.memset(ones_row[:], 1.0)
            # epos: one-hot column at partition pi; pmask: 0 at pi, 1 elsewhere
            epos = spool.tile([128, 1], CDT)
            onec = spool.tile([128, 1], CDT)
            nc.vector.memset(onec[:], 1.0)
            nc.gpsimd.affine_select(
                epos[:], onec[:], pattern=[[0, 1]],
                compare_op=mybir.AluOpType.is_equal, fill=0.0,
                base=-pi, channel_multiplier=1,
            )
            pmask = spool.tile([128, 1], CDT)
            nc.gpsimd.affine_select(
                pmask[:], onec[:], pattern=[[0, 1]],
                compare_op=mybir.AluOpType.not_equal, fill=0.0,
                base=-pi, channel_multiplier=1,
            )

            # ---- q/k/v projections -> (128 hd, 32 pair) columns, fp32 ----
            qT_sb = spool.tile([128, PAIRS], CDT)
            kn_sb = spool.tile([128, PAIRS], CDT)
            vn_sb = spool.tile([128, PAIRS], F32)  # new-v term applied in fp32
            for w_sb, out_sb, ptag in (
                (wq_sb, qT_sb, "pj_a"),
                (wk_sb, kn_sb, "pj_b"),
                (wv_sb, vn_sb, "pj_a"),
            ):
                pj = ps_one.tile([128, PAIRS], F32, tag=ptag)
                for h in range(H_LOC):
                    for c in range(DC):
                        nc.tensor.matmul(
                            pj[:, 16 * h : 16 * (h + 1)],
                            w_sb[:, c, 128 * h : 128 * (h + 1)],
                            xT_sb[:, c, :],
                            start=(c == 0),
                            stop=(c == DC - 1),
                        )
                nc.vector.tensor_copy(out_sb[:], pj[:])

            # ---- attention over pairs ----
            attn_sb = spool.tile([128, PAIRS * kvc], CDT)
            partials = spool.tile([128, PAIRS], F32)
            outU = ps_one.tile([128, PAIRS], F32, tag="outU")
            anew = ps_one.tile([1, PAIRS], F32, tag="anew")

            # ---- per-head epilogue: softmax normalization + new-v term +
            # o_proj (transposed: yT chunks are (128, 16) -> one PSUM bank) ----
            attout = spool.tile([128, PAIRS], CDT)
            yt_ps = [
                ps_one.tile([128, DC, B], F32, tag="yT", name="yt0"),
                ps_one.tile([128, DC, B], F32, tag="pj_b", name="yt1"),
            ]
            yt_sb = spool.tile([128, DC, B], F32)

            def epi(h):
                cs = slice(16 * h, 16 * (h + 1))
                es = ps_one.tile([1, 16], F32, tag="pj_a")
                nc.tensor.matmul(
                    es[:], ones_col[:], partials[:, cs], start=True, stop=True
                )
                recip_h = spool.tile([1, 16], F32, tag=f"recip{h}")
                nc.vector.reciprocal(recip_h[:], es[:])
                anew_h = spool.tile([1, 16], F32, tag=f"anewsb{h}")
                nc.scalar.copy(anew_h[:], anew[:, cs])
                rb = ps_one.tile([128, 16], F32, tag="pj_a")
                nc.tensor.matmul(rb[:], ones_row[:], recip_h[:], start=True, stop=True)
                recip_bc = spool.tile([128, 16], F32, tag=f"rbc{h}")
                nc.scalar.copy(recip_bc[:], rb[:])
                ab2 = ps_one.tile([128, 16], F32, tag="pj_a")
                nc.tensor.matmul(ab2[:], ones_row[:], anew_h[:], start=True, stop=True)
                anew_bc = spool.tile([128, 16], F32, tag=f"abc{h}")
                nc.scalar.copy(anew_bc[:], ab2[:])
                t1 = spool.tile([128, 16], F32, tag=f"t1{h}")
                nc.vector.tensor_tensor(
                    t1[:], vn_sb[:, cs], anew_bc[:], mybir.AluOpType.mult
                )
                t2 = spool.tile([128, 16], F32, tag=f"t2{h}")
                nc.vector.tensor_tensor(t2[:], outU[:, cs], t1[:], mybir.AluOpType.add)
                nc.vector.tensor_tensor(
                    attout[:, cs], t2[:], recip_bc[:], mybir.AluOpType.mult
                )
                for dc in range(DC):
                    nc.tensor.matmul(
                        yt_ps[h][:, dc, :],
                        wo_sb[:, h, 128 * dc : 128 * (dc + 1)],
                        attout[:, cs],
                        start=True,
                        stop=True,
                    )


            def pair_front(p):
                kt = kts.pop(p)
                # overwrite the stale K column at `position` with the new k
                nc.vector.tensor_copy(
                    kt[:, position : position + 1], kn_sb[:, p : p + 1]
                )
                sc = ps_sc.tile([128, kvc], F32, tag="sc")
                for j in range(kvc):
                    nc.tensor.matmul(
                        sc[:, j : j + 1],
                        kt[:, 128 * j : 128 * (j + 1)],
                        qT_sb[:, p : p + 1],
                        start=True,
                        stop=True,
                    )
                ab = attn_sb[:, kvc * p : kvc * (p + 1)]
                nc.scalar.activation(
                    ab,
                    sc[:],
                    mybir.ActivationFunctionType.Exp,
                    scale=SCALE,
                    accum_out=partials[:, p : p + 1],
                )
                # attn weight at `position` -> anew[0, p], then zero it so the
                # stale V row drops out of the V matmuls
                nc.tensor.matmul(
                    anew[:, p : p + 1], epos[:], ab[:, pc : pc + 1],
                    start=True, stop=True,
                )
                nc.vector.tensor_tensor(
                    ab[:, pc : pc + 1], ab[:, pc : pc + 1], pmask[:],
                    mybir.AluOpType.mult,
                )

            def pair_back(p):
                ab = attn_sb[:, kvc * p : kvc * (p + 1)]
                vt = vts.pop(p)
                for j in range(kvc):
                    nc.tensor.matmul(
                        outU[:, p : p + 1],
                        vt[:, j, :],
                        ab[:, j : j + 1],
                        start=(j == 0),
                        stop=(j == kvc - 1),
                    )

            # software-pipelined by one pair: pair p+1's score matmuls are
            # emitted before pair p's V matmuls so the in-order PE stream
            # never stalls on the exp between them
            for p in range(PAIRS):
                if p == 8:
                    nc.gpsimd.dma_start(wo_sb[:], woT)
                if p not in kts:
                    issue_pair_dma(p)
                pair_front(p)
                if p > 0:
                    pair_back(p - 1)
                    if p - 1 == 15:
                        epi(0)
            pair_back(PAIRS - 1)
            epi(H_LOC - 1)
            nc.vector.tensor_copy(yt_sb[:], yt_ps[0][:])
            nc.vector.tensor_tensor(
                yt_sb[:], yt_ps[1][:], yt_sb[:], mybir.AluOpType.add
            )
            nc.sync.dma_start(yT, yt_sb[:])

    nc.compile()
    return nc


def shard_inputs(x, Wq, Wk, Wv, Wo, k_cache, v_cache, prec=PRECISION):
    """Build per-core input maps (head-sharded)."""
    cdt = ml_dtypes.bfloat16 if prec == "bf16" else np.float32
    def sb_layout(a2d, inner):
        # (K*128, inner-layout...) -> (128, K, ...) contiguous per partition
        d0 = a2d.shape[0]
        return np.ascontiguousarray(
            a2d.reshape(d0 // 128, 128, a2d.shape[1]).transpose(1, 0, 2)
        ).astype(cdt)

    x2 = np.ascontiguousarray(np.asarray(x, dtype=np.float32).reshape(B, D))
    xT_full = sb_layout(np.ascontiguousarray(x2.T), B)        # (128, DC, B)
    # K: (H, B, hd, KV); V: (H, B, kv%128, kv//128, hd) partition-swizzled
    kT_all = np.ascontiguousarray(
        np.asarray(k_cache, dtype=np.float32).transpose(1, 0, 3, 2).astype(cdt)
    )
    v_all = np.ascontiguousarray(
        np.asarray(v_cache, dtype=np.float32)
        .reshape(B, H_TOT, KV // 128, 128, HD)
        .transpose(1, 0, 3, 2, 4)
        .astype(cdt)
    )
    Wq = np.asarray(Wq, dtype=np.float32)
    Wk = np.asarray(Wk, dtype=np.float32)
    Wv = np.asarray(Wv, dtype=np.float32)
    Wo = np.asarray(Wo, dtype=np.float32)

    in_maps = []
    for c in range(N_CORES):
        r0, r1 = HS * c, HS * (c + 1)
        in_maps.append(
            {
                "xT": xT_full,
                "wqT": sb_layout(Wq[r0:r1].T, HS),
                "wkT": sb_layout(Wk[r0:r1].T, HS),
                "wvT": sb_layout(Wv[r0:r1].T, HS),
                "woT": sb_layout(Wo[:, r0:r1].T, D),
                "kT": kT_all[H_LOC * c : H_LOC * (c + 1)].reshape(PAIRS, HD, KV),
                "v": v_all[H_LOC * c : H_LOC * (c + 1)].reshape(
                    PAIRS, 128, KV // 128, HD
                ),
            }
        )
    return in_maps


_NC_CACHE = {}


def kernel(x, Wq, Wk, Wv, Wo, k_cache, v_cache, position):
    global LAST_RESULT
    pos = int(position)
    nc = _NC_CACHE.get(pos)
    if nc is None:
        nc = _NC_CACHE[pos] = build_kernel(pos)
    in_maps = shard_inputs(x, Wq, Wk, Wv, Wo, k_cache, v_cache)
    res = run_bass_kernel_spmd(nc, in_maps, core_ids=list(range(N_CORES)))
    LAST_RESULT = res
    out = np.zeros((128, D // 128, B), dtype=np.float32)
    for c in range(N_CORES):
        out += res.results[c]["yT"]
    y2 = out.transpose(1, 0, 2).reshape(D, B)
    return np.ascontiguousarray(y2.T).reshape(B, 1, D)
